# revision 7
# baseline (speedup 1.0000x reference)
"""Trainium2 Bass kernel: EdgeFeatureEncoding scatter-add.

Computes bias[i, j, :] += edge_attr[e] @ W + b over E edges (i, j),
bias shape (N, N, 8) with N = 4096, E = 131072 -> 512 MiB f32 output.

Strategy (8 NeuronCores, SPMD):
- Output rows i are sharded across the 8 cores (512 rows -> 64 MiB each).
- Each core's shard is further split into ZONES (one zero-fill DMA each).
  Host buckets edges per (core, zone) with a fixed per-zone chunk quota so
  the single compiled program fits every core.  Each zone's scatters use a
  dest AP sliced to the zones zeroed so far, so the Tile dependency tracker
  lets zone-r scatters run as soon as zero-fill DMA r completes: zero-fill,
  compute and scatter all pipeline.
- Edge features are shipped pre-transposed ([feat, edge]), so the
  projection is a W-stationary PE matmul (LDWEIGHTS of 8 cols only)
  producing projT [8, 128] per chunk; a cheap PE transpose (8-deep
  contraction) flips it to [128 edges, 8 heads] for the scatter source.
- Edges sharing one (i, j) slot are packed into their zone's leading
  "selection" chunk where the device sums groups with the
  is_equal/selection-matrix matmul trick (duplicate DMA writes then all
  carry the identical group sum).
- Rows [0, 128) of the table are a trash target for padding edges; real
  row d lives at table row 128 + d.  The trash rows are sliced off on the
  host, and keeping them at the START of the table means every sliced
  scatter dest AP still covers them.
"""

import os
from dataclasses import dataclass

import numpy as np

H = 8  # n_heads
F = 128  # edge feature dim
CH = 128  # edges per chunk (one partition tile / one indirect DMA)
TRASH = 128  # trash rows at the START of the table
N_CORES = 8
ZONES = 16  # zero-fill DMAs / scatter zones per core


@dataclass(frozen=True)
class _Cfg:
    n_nodes: int
    n_shards: int
    quota: int  # chunks per zone (first chunk = selection chunk)

    @property
    def rows(self):
        return self.n_nodes // self.n_shards

    @property
    def table_real(self):
        return self.rows * self.n_nodes  # real rows per shard

    @property
    def zone_rows(self):
        return self.table_real // ZONES

    @property
    def table_rows(self):
        return TRASH + self.table_real


_cache: dict = {}


def _build(cfg: _Cfg):
    import concourse.bacc as bacc
    import concourse.bass as bass
    import concourse.mybir as mybir
    import concourse.tile as tile
    from concourse.masks import make_identity

    f32 = mybir.dt.float32
    i32 = mybir.dt.int32
    Q = cfg.quota

    nc = bacc.Bacc(
        "TRN2", target_bir_lowering=False, debug=False, num_devices=cfg.n_shards
    )
    # xt[z, f, c*CH + p] = feature f of edge (zone z, chunk c, row p)
    xt = nc.dram_tensor("xt", [ZONES, F, Q * CH], f32, kind="ExternalInput")
    # idxb[z, p, c] = dest table row of edge (zone z, chunk c, row p)
    idxb = nc.dram_tensor("idxb", [ZONES, CH, Q], i32, kind="ExternalInput")
    w = nc.dram_tensor("w", [F, H], f32, kind="ExternalInput")
    brep = nc.dram_tensor("brep", [CH, H], f32, kind="ExternalInput")
    table = nc.dram_tensor("table", [cfg.table_rows, H], f32, kind="ExternalOutput")

    zcols = cfg.zone_rows * H // 128  # f32 per partition per zero-fill DMA

    with tile.TileContext(nc) as tc:
        with (
            tc.tile_pool(name="const", bufs=1) as constp,
            tc.tile_pool(name="zero", bufs=1) as zerop,
            tc.tile_pool(name="xin", bufs=3) as xinp,
            tc.tile_pool(name="pjt", bufs=4) as pjtp,
            tc.tile_pool(name="small", bufs=4) as smallp,
            tc.tile_pool(name="sel", bufs=2) as selp,
            tc.tile_pool(name="src", bufs=3) as srcp,
            tc.tile_pool(name="psum", bufs=4, space="PSUM") as psp,
        ):
            ident = constp.tile([CH, CH], f32)
            make_identity(nc, ident[:])
            wt = constp.tile([F, H], f32)
            nc.scalar.dma_start(out=wt[:], in_=w.ap())
            bt = constp.tile([CH, H], f32)
            nc.scalar.dma_start(out=bt[:], in_=brep.ap())

            # ---- zero-fill: one big DMA per zone, all on the sync HWDGE ring
            ztile = zerop.tile([128, zcols], f32)
            nc.vector.memset(ztile[:], 0.0)
            zview = table.ap()[TRASH:].rearrange(
                "(c p x) h -> c p (x h)", c=ZONES, p=128
            )
            for z in range(ZONES):
                nc.sync.dma_start(out=zview[z], in_=ztile[:])

            # ---- per-zone edge pipeline
            for z in range(ZONES):
                x_zone = xinp.tile([F, Q * CH], f32, tag="xin")
                nc.scalar.dma_start(out=x_zone[:], in_=xt.ap()[z])
                idx_t = smallp.tile([CH, Q], i32, tag="idx")
                nc.scalar.dma_start(out=idx_t[:], in_=idxb.ap()[z])
                src_t = srcp.tile([CH, Q * H], f32, tag="src")
                # scatters of zone z may start once zones <= z are zeroed
                zdst = table.ap()[: TRASH + (z + 1) * cfg.zone_rows]
                for c in range(Q):
                    # projT [8, 128] = W.T @ xT-chunk  (W stationary: LDW of
                    # 8 columns only)
                    pjt_ps = psp.tile([H, CH], f32, space="PSUM", tag="mm")
                    nc.tensor.matmul(
                        out=pjt_ps[:],
                        lhsT=wt[:],
                        rhs=x_zone[:, c * CH : (c + 1) * CH],
                        start=True,
                        stop=True,
                    )
                    pjt_sb = pjtp.tile([H, CH], f32, tag="pjt")
                    nc.scalar.copy(out=pjt_sb[:], in_=pjt_ps[:])
                    # transpose back: proj [128, 8] (contraction depth 8)
                    pj_ps = psp.tile([CH, H], f32, space="PSUM", tag="tr")
                    nc.tensor.transpose(
                        out=pj_ps[:], in_=pjt_sb[:], identity=ident[:H, :H]
                    )
                    if c == 0:
                        # selection chunk: rows of a duplicate group all get
                        # the group sum
                        pj_sb = smallp.tile([CH, H], f32, tag="pjsb")
                        nc.vector.tensor_add(out=pj_sb[:], in0=pj_ps[:], in1=bt[:])
                        idxf = smallp.tile([CH, 1], f32, tag="idxf")
                        nc.vector.tensor_copy(out=idxf[:], in_=idx_t[:, :1])
                        idt_ps = psp.tile([CH, CH], f32, space="PSUM", tag="mm")
                        nc.tensor.transpose(
                            out=idt_ps[:],
                            in_=idxf[:].to_broadcast([CH, CH]),
                            identity=ident[:],
                        )
                        idt_sb = selp.tile([CH, CH], f32, tag="idt")
                        nc.vector.tensor_copy(out=idt_sb[:], in_=idt_ps[:])
                        selm = selp.tile([CH, CH], f32, tag="selm")
                        nc.vector.tensor_tensor(
                            out=selm[:],
                            in0=idxf[:].to_broadcast([CH, CH]),
                            in1=idt_sb[:],
                            op=mybir.AluOpType.is_equal,
                        )
                        acc_ps = psp.tile([CH, H], f32, space="PSUM", tag="tr")
                        nc.tensor.matmul(
                            out=acc_ps[:], lhsT=selm[:], rhs=pj_sb[:],
                            start=True, stop=True,
                        )
                        nc.vector.tensor_copy(
                            out=src_t[:, :H], in_=acc_ps[:]
                        )
                    else:
                        nc.vector.tensor_add(
                            out=src_t[:, c * H : (c + 1) * H],
                            in0=pj_ps[:],
                            in1=bt[:],
                        )
                    nc.gpsimd.indirect_dma_start(
                        out=zdst,
                        out_offset=bass.IndirectOffsetOnAxis(
                            ap=idx_t[:, c : c + 1], axis=0
                        ),
                        in_=src_t[:, c * H : (c + 1) * H],
                        in_offset=None,
                    )

    nc.compile()
    return nc


def _prepare(edge_index, edge_attr, n_nodes, n_shards):
    """Bucket edges by (shard, zone); sort by dest; pack duplicate groups
    into each zone's selection chunk.  Returns (quota, xt list, idx list)
    where xt is the transposed feature layout [ZONES, F, quota*CH] and idx
    the dest-row layout [ZONES, CH, quota] (table row = TRASH + local slot,
    trash rows < TRASH)."""
    N = n_nodes
    R = N // n_shards
    table_real = R * N
    zone_rows = table_real // ZONES
    i = np.asarray(edge_index[0], dtype=np.int64)
    j = np.asarray(edge_index[1], dtype=np.int64)
    valid = (i >= 0) & (i < N) & (j >= 0) & (j < N)
    eids = np.nonzero(valid)[0]
    i = i[eids]
    j = j[eids]
    shard = i // R
    d = (i - shard * R) * N + j
    zone = d // zone_rows

    edge_attr = np.asarray(edge_attr, dtype=np.float32)

    # per (shard, zone): (sel_e, sel_d, single_e, single_d)
    zones: list = []
    quota = 1
    for s in range(n_shards):
        for z in range(ZONES):
            m = (shard == s) & (zone == z)
            es, ds = eids[m], d[m]
            o = np.argsort(ds, kind="stable")
            es, ds = es[o], ds[o]
            _, start, counts = np.unique(ds, return_index=True, return_counts=True)
            multi = np.nonzero(counts > 1)[0]
            sel_e: list = []
            sel_d: list = []
            for g in multi:
                st, ln = int(start[g]), int(counts[g])
                sel_e.extend(es[st : st + ln].tolist())
                sel_d.extend(ds[st : st + ln].tolist())
            assert len(sel_e) <= CH, (
                f"{len(sel_e)} duplicate-group edges exceed selection chunk"
            )
            single = np.nonzero(counts == 1)[0]
            se, sd = es[start[single]], ds[start[single]]
            zones.append((sel_e, sel_d, se, sd))
            quota = max(quota, 1 + -(-len(se) // CH))

    xs, ids = [], []
    K = quota * CH
    zi = 0
    for s in range(n_shards):
        xt = np.zeros((ZONES, F, quota * CH), np.float32)
        idx = np.zeros((ZONES, CH, quota), np.int32)
        for z in range(ZONES):
            sel_e, sel_d, se, sd = zones[zi]
            zi += 1
            n_sel = len(sel_e)
            be = np.concatenate([np.asarray(sel_e, np.int64), se])
            bd = np.concatenate([np.asarray(sel_d, np.int64), sd])
            # pad: selection chunk to CH, then bucket to K
            pads = [(n_sel, CH - n_sel), (CH + len(se), K - CH - len(se))]
            eb = np.full(K, -1, np.int64)
            db = np.empty(K, np.int64)
            eb[:n_sel] = be[:n_sel]
            db[:n_sel] = TRASH + bd[:n_sel]
            eb[CH : CH + len(se)] = be[n_sel:]
            db[CH : CH + len(se)] = TRASH + bd[n_sel:]
            for at, cnt in pads:
                q = np.arange(cnt)
                db[at : at + cnt] = q % TRASH  # trash rows [0, TRASH)
            real = eb >= 0
            xz = np.zeros((K, F), np.float32)
            xz[real] = edge_attr[eb[real]]
            xt[z] = xz.T  # [F, K]
            idx[z] = db.astype(np.int32).reshape(quota, CH).T  # [CH, quota]
        xs.append(np.ascontiguousarray(xt))
        ids.append(np.ascontiguousarray(idx))
    return quota, xs, ids


LAST_EXEC_NS = None
LAST_RESULTS = None


def kernel(edge_index, edge_attr, num_nodes, W, b):
    from concourse.bass_utils import run_bass_kernel_spmd

    global LAST_EXEC_NS, LAST_RESULTS
    N = int(num_nodes)
    S = N_CORES
    R = N // S
    table_real = R * N

    quota, xs, ids = _prepare(edge_index, edge_attr, N, S)
    cfg = _Cfg(n_nodes=N, n_shards=S, quota=quota)
    nc = _cache.get(cfg)
    if nc is None:
        nc = _build(cfg)
        _cache[cfg] = nc

    W_np = np.ascontiguousarray(np.asarray(W, dtype=np.float32))
    b_rep = np.ascontiguousarray(
        np.broadcast_to(np.asarray(b, dtype=np.float32), (CH, H))
    )
    in_maps = [
        {"xt": xs[s], "idxb": ids[s], "w": W_np, "brep": b_rep} for s in range(S)
    ]
    trace = bool(int(os.environ.get("EDGE_KERNEL_TRACE", "0")))
    res = run_bass_kernel_spmd(nc, in_maps, core_ids=list(range(S)), trace=trace)
    LAST_EXEC_NS = res.exec_time_ns
    LAST_RESULTS = res
    out = np.concatenate(
        [r["table"][TRASH : TRASH + table_real].reshape(R, N, H) for r in res.results],
        axis=0,
    )
    return out


# revision 14
# speedup vs baseline: 2.1671x; 2.1671x over previous
"""Trainium2 Bass kernel: EdgeFeatureEncoding scatter-add (raw bass).

Computes bias[i, j, :] += edge_attr[e] @ W + b over E edges (i, j),
bias shape (N, N, 8) with N = 4096, E = 131072 -> 512 MiB f32 output.

Strategy (8 NeuronCores, SPMD, hand-rolled semaphores):
- Output rows i are sharded across the 8 cores (512 rows -> 64 MiB each).
- Each shard splits into 16 ZONES.  Host buckets edges per (core, zone)
  with a fixed per-zone chunk quota (one compiled program fits all cores).
- One HWDGE ring (sync engine) carries, in FIFO order, the constants then
  interleaved (x_z, zero_z) pairs: edge features stream in just ahead of
  each zone's 4 MiB zero-fill, so compute leads the scatter gate.
- Edge features ship pre-transposed ([feat, edge]); the projection is a
  W-stationary PE matmul (projT [8, 128] per 128-edge chunk) plus a cheap
  8-deep PE transpose back to [128 edges, 8 heads]; DVE adds the bias.
- GpSimd scatters each chunk with one indirect DMA (one dest row per
  partition - HW semantics).  A zone's scatters wait ONLY on that zone's
  own zero-fill semaphore, so zero-fill, compute and scatter pipeline
  with no false serialization.
- Duplicate-destination edges are packed into each zone's chunk 0 and
  group-summed on device via the is_equal/selection-matrix matmul, so
  colliding DMA writes all carry the identical group sum.
- Table rows [0, 128) are a trash target for padding edges (sliced off on
  the host); real row d lives at table row 128 + d.
"""

import os
from dataclasses import dataclass

import numpy as np

H = 8  # n_heads
F = 128  # edge feature dim
CH = 128  # edges per chunk (one partition tile / one indirect DMA)
TRASH = 128  # trash rows at the START of the table
N_CORES = 8
ZONES = 16  # zero-fill DMAs / scatter zones per core


@dataclass(frozen=True)
class _Cfg:
    n_nodes: int
    n_shards: int
    quota: int  # chunks per zone (first chunk = selection chunk)

    @property
    def rows(self):
        return self.n_nodes // self.n_shards

    @property
    def table_real(self):
        return self.rows * self.n_nodes

    @property
    def zone_rows(self):
        return self.table_real // ZONES

    @property
    def table_rows(self):
        return TRASH + self.table_real


_cache: dict = {}


def _build(cfg: _Cfg):
    import concourse.bacc as bacc
    import concourse.bass as bass
    import concourse.mybir as mybir
    from concourse.masks import make_identity

    f32 = mybir.dt.float32
    i32 = mybir.dt.int32
    Q = cfg.quota
    NCH = ZONES * Q  # total chunks

    nc = bacc.Bacc(
        "TRN2", target_bir_lowering=False, debug=False, num_devices=cfg.n_shards
    )
    # xt[z, f, c*CH + p] = feature f of edge (zone z, chunk c, row p)
    xt = nc.dram_tensor("xt", [ZONES, F, Q * CH], f32, kind="ExternalInput")
    # idxb[p, z*Q + c] = dest table row of edge (zone z, chunk c, row p)
    idxb = nc.dram_tensor("idxb", [CH, NCH], i32, kind="ExternalInput")
    w = nc.dram_tensor("w", [F, H], f32, kind="ExternalInput")
    brep = nc.dram_tensor("brep", [CH, H], f32, kind="ExternalInput")
    table = nc.dram_tensor("table", [cfg.table_rows, H], f32, kind="ExternalOutput")

    zcols = cfg.zone_rows * H // 128  # f32 per partition per zero-fill DMA
    zview = table.ap()[TRASH:].rearrange("(z p x) h -> z p (x h)", z=ZONES, p=128)

    # ---- SBUF / PSUM ----
    ztile = nc.alloc_sbuf_tensor("ztile", [128, zcols], f32)
    wt = nc.alloc_sbuf_tensor("wt", [F, H], f32)
    bt = nc.alloc_sbuf_tensor("bt", [CH, H], f32)
    ixt = nc.alloc_sbuf_tensor("ixt", [CH, NCH], i32)
    ident = nc.alloc_sbuf_tensor("ident", [CH, CH], f32)
    xz = [nc.alloc_sbuf_tensor(f"xz{z}", [F, Q * CH], f32) for z in range(ZONES)]
    srcb = nc.alloc_sbuf_tensor("srcb", [CH, NCH * H], f32)
    pjt = [nc.alloc_sbuf_tensor(f"pjt{i}", [H, CH], f32) for i in range(4)]
    idxf = [nc.alloc_sbuf_tensor(f"idxf{i}", [CH, 1], f32) for i in range(2)]
    idt_sb = [nc.alloc_sbuf_tensor(f"idt{i}", [CH, CH], f32) for i in range(2)]
    selm = [nc.alloc_sbuf_tensor(f"selm{i}", [CH, CH], f32) for i in range(2)]
    pj_sb = [nc.alloc_sbuf_tensor(f"pjsb{i}", [CH, H], f32) for i in range(2)]

    mm_ps = [nc.alloc_psum_tensor(f"mm{i}", [H, CH], f32) for i in range(3)]
    tr_ps = [nc.alloc_psum_tensor(f"tr{i}", [CH, H], f32) for i in range(3)]
    idt_ps = nc.alloc_psum_tensor("idtp", [CH, CH], f32)
    acc_ps = nc.alloc_psum_tensor("accp", [CH, H], f32)

    # ---- semaphores ----
    s_zt = nc.alloc_semaphore("s_zt")
    s_w = nc.alloc_semaphore("s_w")
    s_b = nc.alloc_semaphore("s_b")
    s_ix = nc.alloc_semaphore("s_ix")
    s_x = [nc.alloc_semaphore(f"s_x{z}") for z in range(ZONES)]
    s_z = [nc.alloc_semaphore(f"s_z{z}") for z in range(ZONES)]
    s_id = nc.alloc_semaphore("s_id")
    s_mm = nc.alloc_semaphore("s_mm")
    s_cp = nc.alloc_semaphore("s_cp")
    s_tr = nc.alloc_semaphore("s_tr")
    s_src = nc.alloc_semaphore("s_src")
    s_idxf = nc.alloc_semaphore("s_idxf")
    s_idt = nc.alloc_semaphore("s_idt")
    s_idtcp = nc.alloc_semaphore("s_idtcp")
    s_selv = nc.alloc_semaphore("s_selv")
    s_selmm = nc.alloc_semaphore("s_selmm")
    s_sc = nc.alloc_semaphore("s_sc")

    # ---- SYNC: constants, then (x_z, zero_z) interleaved on one ring ----
    sy = nc.sync
    sy.dma_start(out=wt.ap(), in_=w.ap()).then_inc(s_w, 16)
    sy.dma_start(out=bt.ap(), in_=brep.ap()).then_inc(s_b, 16)
    sy.dma_start(out=ixt.ap(), in_=idxb.ap()).then_inc(s_ix, 16)
    sy.wait_ge(s_zt, 1)
    for z in range(ZONES):
        sy.dma_start(out=xz[z].ap(), in_=xt.ap()[z]).then_inc(s_x[z], 16)
        sy.dma_start(out=zview[z], in_=ztile.ap()).then_inc(s_z[z], 16)

    # ---- PE: projection matmul + transpose back (+ selection matmuls) ----
    pe = nc.tensor
    pe.wait_ge(s_w, 16)
    pe.wait_ge(s_id, 2)
    for n in range(NCH):
        z, c = divmod(n, Q)
        if c == 0:
            pe.wait_ge(s_x[z], 16)
        if n >= 3:
            pe.wait_ge(s_cp, n - 2)  # mm_ps slot n%3 drained by ACT
        pe.matmul(
            out=mm_ps[n % 3].ap(),
            lhsT=wt.ap(),
            rhs=xz[z].ap()[:, c * CH : (c + 1) * CH],
            start=True,
            stop=True,
        ).then_inc(s_mm, 1)
        pe.wait_ge(s_cp, n + 1)  # pjt[n%4] written by ACT
        if n >= 3:
            pe.wait_ge(s_src, n - 2)  # tr_ps slot n%3 drained by DVE
        pe.transpose(
            out=tr_ps[n % 3].ap(),
            in_=pjt[n % 4].ap(),
            identity=ident.ap()[:H, :H],
        ).then_inc(s_tr, 1)
        if c == 0:
            pe.wait_ge(s_idxf, z + 1)
            if z >= 1:
                pe.wait_ge(s_idtcp, z)  # idt_ps drained by DVE
            pe.transpose(
                out=idt_ps.ap(),
                in_=idxf[z % 2].ap().to_broadcast([CH, CH]),
                identity=ident.ap(),
            ).then_inc(s_idt, 1)
            pe.wait_ge(s_selv, 2 * (z + 1))  # selm + biased proj ready
            if z >= 1:
                pe.wait_ge(s_src, (z - 1) * Q + 1)  # acc_ps drained by DVE
            pe.matmul(
                out=acc_ps.ap(),
                lhsT=selm[z % 2].ap(),
                rhs=pj_sb[z % 2].ap(),
                start=True,
                stop=True,
            ).then_inc(s_selmm, 1)

    # ---- ACT: PSUM->SBUF copies of projT ----
    ac = nc.scalar
    for n in range(NCH):
        ac.wait_ge(s_mm, n + 1)
        if n >= 4:
            ac.wait_ge(s_tr, n - 3)  # pjt slot n%4 consumed by PE transpose
        ac.copy(out=pjt[n % 4].ap(), in_=mm_ps[n % 3].ap()).then_inc(s_cp, 1)

    # ---- DVE: ztile memset, bias adds, selection machinery ----
    dv = nc.vector
    dv.memset(ztile.ap(), 0.0).then_inc(s_zt, 1)
    dv.wait_ge(s_b, 16)
    dv.wait_ge(s_ix, 16)
    for n in range(NCH):
        z, c = divmod(n, Q)
        dv.wait_ge(s_tr, n + 1)
        if c == 0:
            dv.tensor_add(
                out=pj_sb[z % 2].ap(), in0=tr_ps[n % 3].ap(), in1=bt.ap()
            ).then_inc(s_selv, 1)
            dv.tensor_copy(out=idxf[z % 2].ap(), in_=ixt.ap()[:, n : n + 1]).then_inc(
                s_idxf, 1
            )
            dv.wait_ge(s_idt, z + 1)
            dv.tensor_copy(out=idt_sb[z % 2].ap(), in_=idt_ps.ap()).then_inc(
                s_idtcp, 1
            )
            dv.wait_ge(s_idtcp, z + 1)  # own-pipe drain before reading idt_sb
            dv.wait_ge(s_idxf, z + 1)
            dv.tensor_tensor(
                out=selm[z % 2].ap(),
                in0=idxf[z % 2].ap().to_broadcast([CH, CH]),
                in1=idt_sb[z % 2].ap(),
                op=mybir.AluOpType.is_equal,
            ).then_inc(s_selv, 1)
            dv.wait_ge(s_selmm, z + 1)
            dv.tensor_copy(
                out=srcb.ap()[:, n * H : (n + 1) * H], in_=acc_ps.ap()
            ).then_inc(s_src, 1)
        else:
            dv.tensor_add(
                out=srcb.ap()[:, n * H : (n + 1) * H],
                in0=tr_ps[n % 3].ap(),
                in1=bt.ap(),
            ).then_inc(s_src, 1)

    # ---- POOL: identity build, then one indirect scatter per chunk ----
    gp = nc.gpsimd
    gp.memset(ident.ap(), 0.0).then_inc(s_id, 1)
    gp.wait_ge(s_id, 1)
    gp.affine_select(
        out=ident.ap(),
        in_=ident.ap(),
        compare_op=mybir.AluOpType.not_equal,
        fill=1.0,
        base=0,
        pattern=[[-1, CH]],
        channel_multiplier=1,
    ).then_inc(s_id, 1)  # s_id reaches 2 when identity is ready
    gp.wait_ge(s_ix, 16)
    for n in range(NCH):
        z, c = divmod(n, Q)
        if c == 0:
            gp.wait_ge(s_z[z], 16)  # this zone's rows are zeroed
        gp.wait_ge(s_src, n + 1)
        gp.indirect_dma_start(
            out=table.ap(),
            out_offset=bass.IndirectOffsetOnAxis(ap=ixt.ap()[:, n : n + 1], axis=0),
            in_=srcb.ap()[:, n * H : (n + 1) * H],
            in_offset=None,
        ).then_inc(s_sc, 16)
    gp.wait_ge(s_sc, 16 * NCH)

    nc.compile()
    return nc


def _prepare(edge_index, edge_attr, n_nodes, n_shards):
    """Bucket edges by (shard, zone); duplicate-dest groups go to each
    zone's chunk 0.  Returns (quota, xt list [ZONES, F, quota*CH],
    idx list [CH, ZONES*quota]); table row = TRASH + local slot, trash
    rows < TRASH."""
    N = n_nodes
    R = N // n_shards
    table_real = R * N
    zone_rows = table_real // ZONES
    i = np.asarray(edge_index[0], dtype=np.int64)
    j = np.asarray(edge_index[1], dtype=np.int64)
    valid = (i >= 0) & (i < N) & (j >= 0) & (j < N)
    eids = np.nonzero(valid)[0]
    i = i[eids]
    j = j[eids]
    shard = i // R
    d = (i - shard * R) * N + j
    zone = d // zone_rows

    edge_attr = np.asarray(edge_attr, dtype=np.float32)

    zones: list = []
    quota = 1
    for s in range(n_shards):
        for z in range(ZONES):
            m = (shard == s) & (zone == z)
            es, ds = eids[m], d[m]
            o = np.argsort(ds, kind="stable")
            es, ds = es[o], ds[o]
            _, start, counts = np.unique(ds, return_index=True, return_counts=True)
            multi = np.nonzero(counts > 1)[0]
            sel_e: list = []
            sel_d: list = []
            for g in multi:
                st, ln = int(start[g]), int(counts[g])
                sel_e.extend(es[st : st + ln].tolist())
                sel_d.extend(ds[st : st + ln].tolist())
            assert len(sel_e) <= CH, (
                f"{len(sel_e)} duplicate-group edges exceed selection chunk"
            )
            single = np.nonzero(counts == 1)[0]
            se, sd = es[start[single]], ds[start[single]]
            zones.append((sel_e, sel_d, se, sd))
            quota = max(quota, 1 + -(-len(se) // CH))

    xs, ids = [], []
    K = quota * CH
    zi = 0
    for s in range(n_shards):
        xt = np.zeros((ZONES, F, K), np.float32)
        idx = np.zeros((ZONES, K), np.int64)
        for z in range(ZONES):
            sel_e, sel_d, se, sd = zones[zi]
            zi += 1
            n_sel = len(sel_e)
            eb = np.full(K, -1, np.int64)
            db = np.arange(K, dtype=np.int64) % TRASH  # default: trash rows
            eb[:n_sel] = sel_e
            db[:n_sel] = TRASH + np.asarray(sel_d, np.int64)
            eb[CH : CH + len(se)] = se
            db[CH : CH + len(se)] = TRASH + sd
            real = eb >= 0
            xzn = np.zeros((K, F), np.float32)
            xzn[real] = edge_attr[eb[real]]
            xt[z] = xzn.T
            idx[z] = db
        # idxb layout [CH, ZONES*quota]: [p, z*quota + c]
        ib = idx.reshape(ZONES, quota, CH).transpose(2, 0, 1).reshape(CH, -1)
        xs.append(np.ascontiguousarray(xt))
        ids.append(np.ascontiguousarray(ib.astype(np.int32)))
    return quota, xs, ids


LAST_EXEC_NS = None
LAST_RESULTS = None


def kernel(edge_index, edge_attr, num_nodes, W, b):
    from concourse.bass_utils import run_bass_kernel_spmd

    global LAST_EXEC_NS, LAST_RESULTS
    N = int(num_nodes)
    S = N_CORES
    R = N // S
    table_real = R * N

    quota, xs, ids = _prepare(edge_index, edge_attr, N, S)
    cfg = _Cfg(n_nodes=N, n_shards=S, quota=quota)
    nc = _cache.get(cfg)
    if nc is None:
        nc = _build(cfg)
        _cache[cfg] = nc

    W_np = np.ascontiguousarray(np.asarray(W, dtype=np.float32))
    b_rep = np.ascontiguousarray(
        np.broadcast_to(np.asarray(b, dtype=np.float32), (CH, H))
    )
    in_maps = [
        {"xt": xs[s], "idxb": ids[s], "w": W_np, "brep": b_rep} for s in range(S)
    ]
    trace = bool(int(os.environ.get("EDGE_KERNEL_TRACE", "0")))
    res = run_bass_kernel_spmd(nc, in_maps, core_ids=list(range(S)), trace=trace)
    LAST_EXEC_NS = res.exec_time_ns
    LAST_RESULTS = res
    out = np.concatenate(
        [r["table"][TRASH : TRASH + table_real].reshape(R, N, H) for r in res.results],
        axis=0,
    )
    return out


# revision 15
# speedup vs baseline: 2.8302x; 1.3060x over previous
"""Trainium2 Bass kernel: EdgeFeatureEncoding scatter-add (raw bass).

Computes bias[i, j, :] += edge_attr[e] @ W + b over E edges (i, j),
bias shape (N, N, 8) with N = 4096, E = 131072 -> 512 MiB f32 output.

Strategy (8 NeuronCores, SPMD, hand-rolled semaphores):
- Output rows i are sharded across the 8 cores (512 rows -> 64 MiB each).
- Each shard splits into 16 ZONES with per-zone chunk counts (max over
  cores, so one compiled program fits all cores).
- One HWDGE ring (sync engine) carries, in FIFO order, the constants then
  interleaved (x_z, zero_z) transfers: edge features stream in just ahead
  of each zone's zero-fill, so compute leads the scatter gate.
- Edge features ship pre-transposed ([feat, edge]), so the projection is
  ONE PE matmul per 128-edge chunk straight into [128 edges, 8 heads]
  PSUM (lhsT = xT chunk, rhs = W); DVE adds the bias into the scatter
  source buffer.  No transposes, no PSUM->SBUF relays.
- GpSimd scatters each chunk with one indirect DMA (one dest row per
  partition - HW semantics).  A zone's scatters wait ONLY on that zone's
  own zero-fill semaphore: zero-fill, compute and scatter all pipeline.
- Each zone's chunk 0 carries every duplicate-destination group (plus
  singleton filler); the device group-sums it with the
  is_equal/selection-matrix matmul, so colliding DMA writes all carry the
  identical group sum (singletons pass through the selection matmul
  unchanged).
- Table rows [0, 128) are a trash target for padding edges (sliced off on
  the host); real row d lives at table row 128 + d.
"""

import os
from dataclasses import dataclass

import numpy as np

H = 8  # n_heads
F = 128  # edge feature dim
CH = 128  # edges per chunk (one partition tile / one indirect DMA)
TRASH = 128  # trash rows at the START of the table
N_CORES = 8
ZONES = 16  # zero-fill zones per core
ZSPLIT = 2  # zero-fill DMAs per zone


@dataclass(frozen=True)
class _Cfg:
    n_nodes: int
    n_shards: int
    quotas: tuple  # chunks per zone (chunk 0 of each zone = selection chunk)

    @property
    def rows(self):
        return self.n_nodes // self.n_shards

    @property
    def table_real(self):
        return self.rows * self.n_nodes

    @property
    def zone_rows(self):
        return self.table_real // ZONES

    @property
    def table_rows(self):
        return TRASH + self.table_real


_cache: dict = {}


def _build(cfg: _Cfg):
    import concourse.bacc as bacc
    import concourse.bass as bass
    import concourse.mybir as mybir

    f32 = mybir.dt.float32
    i32 = mybir.dt.int32
    quotas = cfg.quotas
    NCH = sum(quotas)  # total chunks
    ofs = [0]
    for q in quotas:
        ofs.append(ofs[-1] + q)

    nc = bacc.Bacc(
        "TRN2", target_bir_lowering=False, debug=False, num_devices=cfg.n_shards
    )
    # xt[f, (ofs[z] + c)*CH + p] = feature f of edge (zone z, chunk c, row p)
    xt = nc.dram_tensor("xt", [F, NCH * CH], f32, kind="ExternalInput")
    # idxb[p, ofs[z] + c] = dest table row of edge (zone z, chunk c, row p)
    idxb = nc.dram_tensor("idxb", [CH, NCH], i32, kind="ExternalInput")
    w = nc.dram_tensor("w", [F, H], f32, kind="ExternalInput")
    brep = nc.dram_tensor("brep", [CH, H], f32, kind="ExternalInput")
    table = nc.dram_tensor("table", [cfg.table_rows, H], f32, kind="ExternalOutput")

    zcols = cfg.zone_rows * H // (128 * ZSPLIT)  # f32/partition per zero DMA
    zview = table.ap()[TRASH:].rearrange(
        "(zz p x) h -> zz p (x h)", zz=ZONES * ZSPLIT, p=128
    )

    # ---- SBUF / PSUM ----
    ztile = nc.alloc_sbuf_tensor("ztile", [128, zcols], f32)
    wt = nc.alloc_sbuf_tensor("wt", [F, H], f32)
    bt = nc.alloc_sbuf_tensor("bt", [CH, H], f32)
    ixt = nc.alloc_sbuf_tensor("ixt", [CH, NCH], i32)
    ident = nc.alloc_sbuf_tensor("ident", [CH, CH], f32)
    xz = [
        nc.alloc_sbuf_tensor(f"xz{z}", [F, quotas[z] * CH], f32) for z in range(ZONES)
    ]
    srcb = nc.alloc_sbuf_tensor("srcb", [CH, NCH * H], f32)
    idxf = [nc.alloc_sbuf_tensor(f"idxf{i}", [CH, 1], f32) for i in range(2)]
    idt_sb = [nc.alloc_sbuf_tensor(f"idt{i}", [CH, CH], f32) for i in range(2)]
    selm = [nc.alloc_sbuf_tensor(f"selm{i}", [CH, CH], f32) for i in range(2)]
    pj_sb = [nc.alloc_sbuf_tensor(f"pjsb{i}", [CH, H], f32) for i in range(2)]

    pj_ps = [nc.alloc_psum_tensor(f"pj{i}", [CH, H], f32) for i in range(4)]
    idt_ps = nc.alloc_psum_tensor("idtp", [CH, CH], f32)
    acc_ps = nc.alloc_psum_tensor("accp", [CH, H], f32)

    # ---- semaphores ----
    s_zt = nc.alloc_semaphore("s_zt")
    s_w = nc.alloc_semaphore("s_w")
    s_b = nc.alloc_semaphore("s_b")
    s_ix = nc.alloc_semaphore("s_ix")
    s_x = [nc.alloc_semaphore(f"s_x{z}") for z in range(ZONES)]
    s_z = [nc.alloc_semaphore(f"s_z{z}") for z in range(ZONES)]
    s_id = nc.alloc_semaphore("s_id")
    s_mm = nc.alloc_semaphore("s_mm")
    s_src = nc.alloc_semaphore("s_src")
    s_idxf = nc.alloc_semaphore("s_idxf")
    s_idt = nc.alloc_semaphore("s_idt")
    s_idtcp = nc.alloc_semaphore("s_idtcp")
    s_selv = nc.alloc_semaphore("s_selv")
    s_selmm = nc.alloc_semaphore("s_selmm")
    s_sc = nc.alloc_semaphore("s_sc")

    # ---- SYNC: constants, then (x_z, zero_z) interleaved on one ring ----
    sy = nc.sync
    sy.dma_start(out=wt.ap(), in_=w.ap()).then_inc(s_w, 16)
    sy.dma_start(out=bt.ap(), in_=brep.ap()).then_inc(s_b, 16)
    sy.dma_start(out=ixt.ap(), in_=idxb.ap()).then_inc(s_ix, 16)
    sy.wait_ge(s_zt, 1)
    for z in range(ZONES):
        sy.dma_start(
            out=xz[z].ap(), in_=xt.ap()[:, ofs[z] * CH : ofs[z + 1] * CH]
        ).then_inc(s_x[z], 16)
        for v in range(ZSPLIT):
            sy.dma_start(out=zview[z * ZSPLIT + v], in_=ztile.ap()).then_inc(
                s_z[z], 16
            )

    # ---- PE: one projection matmul per chunk (+ selection matmuls) ----
    pe = nc.tensor
    pe.wait_ge(s_w, 16)
    pe.wait_ge(s_id, 2)
    n = 0
    for z in range(ZONES):
        for c in range(quotas[z]):
            if c == 0:
                pe.wait_ge(s_x[z], 16)
            if n >= 4:
                pe.wait_ge(s_src, n - 3)  # pj_ps slot n%4 drained by DVE
            pe.matmul(
                out=pj_ps[n % 4].ap(),
                lhsT=xz[z].ap()[:, c * CH : (c + 1) * CH],
                rhs=wt.ap(),
                start=True,
                stop=True,
            ).then_inc(s_mm, 1)
            if c == 0:
                pe.wait_ge(s_idxf, z + 1)
                if z >= 1:
                    pe.wait_ge(s_idtcp, z)  # idt_ps drained by DVE
                pe.transpose(
                    out=idt_ps.ap(),
                    in_=idxf[z % 2].ap().to_broadcast([CH, CH]),
                    identity=ident.ap(),
                ).then_inc(s_idt, 1)
                pe.wait_ge(s_selv, 2 * (z + 1))  # selm + biased proj ready
                if z >= 1:
                    pe.wait_ge(s_src, ofs[z - 1] + 1)  # acc_ps drained by DVE
                pe.matmul(
                    out=acc_ps.ap(),
                    lhsT=selm[z % 2].ap(),
                    rhs=pj_sb[z % 2].ap(),
                    start=True,
                    stop=True,
                ).then_inc(s_selmm, 1)
            n += 1

    # ---- DVE: ztile memset, bias adds, selection machinery ----
    dv = nc.vector
    dv.memset(ztile.ap(), 0.0).then_inc(s_zt, 1)
    dv.wait_ge(s_b, 16)
    dv.wait_ge(s_ix, 16)
    n = 0
    for z in range(ZONES):
        for c in range(quotas[z]):
            dv.wait_ge(s_mm, n + 1)
            if c == 0:
                dv.tensor_add(
                    out=pj_sb[z % 2].ap(), in0=pj_ps[n % 4].ap(), in1=bt.ap()
                ).then_inc(s_selv, 1)
                dv.tensor_copy(
                    out=idxf[z % 2].ap(), in_=ixt.ap()[:, n : n + 1]
                ).then_inc(s_idxf, 1)
                dv.wait_ge(s_idt, z + 1)
                dv.tensor_copy(out=idt_sb[z % 2].ap(), in_=idt_ps.ap()).then_inc(
                    s_idtcp, 1
                )
                dv.wait_ge(s_idtcp, z + 1)  # own-pipe drain before reading
                dv.wait_ge(s_idxf, z + 1)
                dv.tensor_tensor(
                    out=selm[z % 2].ap(),
                    in0=idxf[z % 2].ap().to_broadcast([CH, CH]),
                    in1=idt_sb[z % 2].ap(),
                    op=mybir.AluOpType.is_equal,
                ).then_inc(s_selv, 1)
                dv.wait_ge(s_selmm, z + 1)
                dv.tensor_copy(
                    out=srcb.ap()[:, n * H : (n + 1) * H], in_=acc_ps.ap()
                ).then_inc(s_src, 1)
            else:
                dv.tensor_add(
                    out=srcb.ap()[:, n * H : (n + 1) * H],
                    in0=pj_ps[n % 4].ap(),
                    in1=bt.ap(),
                ).then_inc(s_src, 1)
            n += 1

    # ---- POOL: identity build, then one indirect scatter per chunk ----
    gp = nc.gpsimd
    gp.memset(ident.ap(), 0.0).then_inc(s_id, 1)
    gp.wait_ge(s_id, 1)
    gp.affine_select(
        out=ident.ap(),
        in_=ident.ap(),
        compare_op=mybir.AluOpType.not_equal,
        fill=1.0,
        base=0,
        pattern=[[-1, CH]],
        channel_multiplier=1,
    ).then_inc(s_id, 1)  # s_id == 2 -> identity ready
    gp.wait_ge(s_ix, 16)
    n = 0
    for z in range(ZONES):
        for c in range(quotas[z]):
            if c == 0:
                gp.wait_ge(s_z[z], 16 * ZSPLIT)  # this zone's rows are zeroed
            gp.wait_ge(s_src, n + 1)
            gp.indirect_dma_start(
                out=table.ap(),
                out_offset=bass.IndirectOffsetOnAxis(
                    ap=ixt.ap()[:, n : n + 1], axis=0
                ),
                in_=srcb.ap()[:, n * H : (n + 1) * H],
                in_offset=None,
            ).then_inc(s_sc, 16)
            n += 1
    gp.wait_ge(s_sc, 16 * NCH)

    nc.compile()
    return nc


def _prepare(edge_index, edge_attr, n_nodes, n_shards):
    """Bucket edges by (shard, zone).  Chunk 0 of each zone = all
    duplicate-dest groups + singleton filler; remaining singles fill
    chunks 1..  Returns (quotas, xt list [F, NCH*CH], idx list [CH, NCH])
    with per-zone chunk counts maxed over cores.  Table row = TRASH +
    local slot; trash rows < TRASH."""
    N = n_nodes
    R = N // n_shards
    table_real = R * N
    zone_rows = table_real // ZONES
    i = np.asarray(edge_index[0], dtype=np.int64)
    j = np.asarray(edge_index[1], dtype=np.int64)
    valid = (i >= 0) & (i < N) & (j >= 0) & (j < N)
    eids = np.nonzero(valid)[0]
    i = i[eids]
    j = j[eids]
    shard = i // R
    d = (i - shard * R) * N + j
    zone = d // zone_rows

    edge_attr = np.asarray(edge_attr, dtype=np.float32)

    buckets: list = []  # (s, z) -> (edges, dests) ordered: groups then singles
    counts_per_zone = np.zeros((n_shards, ZONES), np.int64)
    for s in range(n_shards):
        for z in range(ZONES):
            m = (shard == s) & (zone == z)
            es, ds = eids[m], d[m]
            o = np.argsort(ds, kind="stable")
            es, ds = es[o], ds[o]
            _, start, counts = np.unique(ds, return_index=True, return_counts=True)
            multi = np.nonzero(counts > 1)[0]
            gsel = np.concatenate(
                [np.arange(start[g], start[g] + counts[g]) for g in multi]
            ) if len(multi) else np.empty(0, np.int64)
            n_grp = len(gsel)
            assert n_grp <= CH, f"{n_grp} duplicate-group edges exceed chunk 0"
            ssel = start[counts == 1]
            order = np.concatenate([gsel, ssel]).astype(np.int64)
            buckets.append((es[order], ds[order]))
            counts_per_zone[s, z] = len(order)

    # per-zone chunk count: maxed over cores (>=1; chunk 0 always exists)
    quotas = tuple(
        int(max(1, -(-int(counts_per_zone[:, z].max()) // CH)))
        for z in range(ZONES)
    )
    NCH = sum(quotas)
    ofs = [0]
    for q in quotas:
        ofs.append(ofs[-1] + q)

    xs, ids = [], []
    bi = 0
    for s in range(n_shards):
        xtp = np.zeros((F, NCH * CH), np.float32)
        idx = np.empty(NCH * CH, np.int64)
        idx[:] = np.arange(NCH * CH) % TRASH  # default: trash rows
        for z in range(ZONES):
            be, bd = buckets[bi]
            bi += 1
            at = ofs[z] * CH
            ne = len(be)
            idx[at : at + ne] = TRASH + bd
            xtp[:, at : at + ne] = edge_attr[be].T
        xs.append(np.ascontiguousarray(xtp))
        ids.append(
            np.ascontiguousarray(
                idx.reshape(NCH, CH).T.astype(np.int32)
            )  # [p, n]
        )
    return quotas, xs, ids


LAST_EXEC_NS = None
LAST_RESULTS = None


def kernel(edge_index, edge_attr, num_nodes, W, b):
    from concourse.bass_utils import run_bass_kernel_spmd

    global LAST_EXEC_NS, LAST_RESULTS
    N = int(num_nodes)
    S = N_CORES
    R = N // S
    table_real = R * N

    quotas, xs, ids = _prepare(edge_index, edge_attr, N, S)
    cfg = _Cfg(n_nodes=N, n_shards=S, quotas=quotas)
    nc = _cache.get(cfg)
    if nc is None:
        nc = _build(cfg)
        _cache[cfg] = nc

    W_np = np.ascontiguousarray(np.asarray(W, dtype=np.float32))
    b_rep = np.ascontiguousarray(
        np.broadcast_to(np.asarray(b, dtype=np.float32), (CH, H))
    )
    in_maps = [
        {"xt": xs[s], "idxb": ids[s], "w": W_np, "brep": b_rep} for s in range(S)
    ]
    trace = bool(int(os.environ.get("EDGE_KERNEL_TRACE", "0")))
    res = run_bass_kernel_spmd(nc, in_maps, core_ids=list(range(S)), trace=trace)
    LAST_EXEC_NS = res.exec_time_ns
    LAST_RESULTS = res
    out = np.concatenate(
        [r["table"][TRASH : TRASH + table_real].reshape(R, N, H) for r in res.results],
        axis=0,
    )
    return out
